# revision 35
# baseline (speedup 1.0000x reference)
"""Trainium2 Bass kernel for nn_ObjectLoss (YOLO-style objectness BCE loss).

Reference semantics (per scale s with grid G):
    pred = out_s[..., 4]                            # objectness channel
    per-target best anchor by IoU of (w,h) boxes; cells (b, a*, ty*G, tx*G)
    with iou > 0.5 get gt=1 (idempotent scatter)
    loss_s = mean(-(gt*log(p) + (1-gt)*log1p(-p)))
    loss = sum over 3 scales

Strategy (8 cores, data-parallel over batch, 2 batches/core):
  - Only channel 4 of 85 is ever needed: gather it with strided DMA
    (1/85th of the bytes).  The gather is SDMA-packet-bound: HWDGE emits
    one packet per 4B descriptor (~11.5 ns/pkt/engine), while SWDGE
    (gpsimd) aggregates ~57 descriptors per packet (~3.5 ns/desc of
    SDMA engine time).  So the big scale rides SWDGE and a ~6k-desc
    slice rides the sync HWDGE ring to use leftover SDMA capacity
    (SWDGE generation, at ~0.41 ns/desc, is the serial bottleneck).
  - One gather DMA per scale; rows (b, a, h) are packed into column
    chunks of P partitions so each scale lands in a single SBUF tile:
    s0 [96, 4*64], s1 [96, 2*32], s2 [48, 2*16].
  - gt grid built on-device without scatter: one-hot(row) x one-hot(col)
    outer products accumulated over targets == one matmul per 128-row
    chunk into a per-scale PSUM tile.
  - BCE = -sum(L1) + sum(gt*(L1-L2)) with L1=ln(1-p), L2=ln(p), computed
    per scale with ACT-engine Ln + fused accumulators; per-core partial
    sums are reduced on host (psum of per-shard sums).

Hardware note: each compute instruction can encode only ONE semaphore
wait, so the program is shaped to give every instruction at most one
unobserved cross-engine dependency: all small inputs ride in a single
"consts" DMA, each engine touches it early, and psum-consuming ops are
split so they wait only on the PE semaphore.
"""

import os
import sys

import numpy as np

for _p in ("/opt/trn_rl_repo", "/root/.axon_site/_ro/trn_rl_repo"):
    if os.path.isdir(_p) and _p not in sys.path:
        sys.path.insert(0, _p)
        break

GS = (64, 32, 16)  # grid size per scale (H == W)
B, A, T, C = 16, 3, 64, 85
NCORES = 8
BL = B // NCORES  # batches per core
OBJ = 4  # objectness channel

# per-scale packing: rows (b a h) -> (chunk, partition); chunks never
# cross batch boundaries (matmul out base partition must be 0)
PPART = (96, 96, 48)  # partitions per chunk
NCHUNK = (4, 2, 2)  # chunks per scale (NCHUNK*PPART == BL*A*g)

# consts layout [128, NCONST]: per-scale iota repeated 4x, anchors
# (replicated across partitions), targets re-laid-out as [t, (b k)],
# a ones column and a zeros column (activation bias operands).
IOTA_OFF = []
_off = 0
for _g in GS:
    IOTA_OFF.append(_off)
    _off += 4 * _g
ANC_OFF = _off          # 18 cols: (s, a, d)
TGT_OFF = _off + 18     # 10 cols: (b, k), rows = t
ONE_OFF = TGT_OFF + 10  # 1.0
ZERO_OFF = ONE_OFF + 1  # 0.0
NCONST = ZERO_OFF + 1

_CONST_BASE = None


def _const_base():
    global _CONST_BASE
    if _CONST_BASE is None:
        c = np.zeros((128, NCONST), np.float32)
        for s, g in enumerate(GS):
            c[:, IOTA_OFF[s] : IOTA_OFF[s] + 4 * g] = np.tile(
                np.arange(g, dtype=np.float32), 4
            )[None, :]
        c[:, ONE_OFF] = 1.0
        _CONST_BASE = c
    return _CONST_BASE


_BUILT = None


def _build():
    """Build the SPMD bass program (same program on all 8 cores)."""
    global _BUILT
    if _BUILT is not None:
        return _BUILT

    from contextlib import ExitStack

    import concourse.bass as bass
    import concourse.tile as tile
    from concourse import mybir

    f32 = mybir.dt.float32
    Alu = mybir.AluOpType
    Act = mybir.ActivationFunctionType

    nc = bass.Bass()
    d_outs = [
        nc.declare_dram_parameter(f"out{s}", [BL, A, g, g, C], f32, isOutput=False)
        for s, g in enumerate(GS)
    ]
    d_const = nc.declare_dram_parameter("consts", [128, NCONST], f32, isOutput=False)
    # per-chunk, per-partition partial sums: cols (chunk, {gg, l1})
    d_part = nc.declare_dram_parameter("partial", [96, 16], f32, isOutput=True)

    with tile.TileContext(nc) as tc, ExitStack() as ctx:
        sb = ctx.enter_context(tc.tile_pool(name="sb", bufs=1))
        ps = ctx.enter_context(tc.tile_pool(name="ps", bufs=1, space="PSUM"))

        # ---------- gathers first: no deps, start the SDMA engines ----------
        # objectness channel of scale s as [chunk, partition, w]; the
        # dest AP iterates (c, p, w) to match the row-major src rows.
        pred = []
        for s, g in enumerate(GS):
            P, NCH = PPART[s], NCHUNK[s]
            t = sb.tile([P, NCH * g], f32, tag=f"pred{s}")
            pred.append(t)

        def gather(eng, s):
            g = GS[s]
            P, NCH = PPART[s], NCHUNK[s]
            rows = d_outs[s][:].rearrange("b a h w c -> (b a h) w c")
            with nc.allow_non_contiguous_dma("objectness channel gather"):
                for cidx in range(NCH):
                    src = rows[cidx * P : (cidx + 1) * P, :, OBJ]
                    dst = pred[s][:, cidx * g : (cidx + 1) * g]
                    eng.dma_start(out=dst, in_=src)

        # consts first (everything on the vector engine needs it), then the
        # gathers, biggest scale first so its BCE overlaps later gathers.
        # All on sync HWDGE: the gather is bound at ~11.5 ns per 4B
        # descriptor per SDMA engine (HBM-read issue cost); neither SWDGE
        # aggregation nor a second DGE ring changes that, and SWDGE adds
        # generation stalls on top.
        # consts on the scalar (ACT) HWDGE ring: off the sync ring so the
        # gathers own it entirely, off gpsimd so the 8 ak out-DMAs don't
        # exceed the 8 SWDGE sem lanes (a 9th forces a lane-reuse guard
        # wait that walrus can't encode alongside the data dep)
        consts = sb.tile([128, NCONST], f32, tag="consts")
        nc.scalar.dma_start(out=consts[:], in_=d_const[:])
        gather(nc.sync, 0)     # 24576 descs
        gather(nc.sync, 1)     # 6144 descs
        gather(nc.sync, 2)     # 1536 descs

        # ACT warm-up touch of consts so later activations never need a
        # consts wait (one sem wait max per instruction).
        warm = sb.tile([1, 1], f32, tag="warm")
        nc.scalar.copy(warm[:], consts[0:1, 0:1])

        ancb = consts[0:64, ANC_OFF : ANC_OFF + 18]  # (s, a, d)
        tgt = consts[0:64, TGT_OFF : TGT_OFF + 10]  # rows=t, cols=(b, k)

        # ---------- per-target math (all [64, *] tiles; partition = t) ----------
        tgt_kb = tgt.rearrange("p (b k) -> p k b", b=BL)  # [64, 5, BL]
        xsel = tgt_kb[:, 1:3, :]  # (tx, ty) per b
        wsel = tgt_kb[:, 3:5, :]  # (tw, th) per b

        x4 = sb.tile([64, 12], f32, tag="x4")  # (s, dir, b): x*G
        x4m1 = sb.tile([64, 12], f32, tag="x4m1")  # x*G - 1
        twth = sb.tile([64, 12], f32, tag="twth")  # (s, d, b): box wh in grid units
        for s, g in enumerate(GS):
            o = x4[:, 4 * s : 4 * s + 4].rearrange("p (k b) -> p k b", k=2)
            nc.vector.tensor_scalar(
                out=o, in0=xsel, scalar1=float(g), scalar2=None, op0=Alu.mult
            )
            o = x4m1[:, 4 * s : 4 * s + 4].rearrange("p (k b) -> p k b", k=2)
            nc.vector.tensor_scalar(
                out=o,
                in0=xsel,
                scalar1=float(g),
                scalar2=1.0,
                op0=Alu.mult,
                op1=Alu.subtract,
            )
            o = twth[:, 4 * s : 4 * s + 4].rearrange("p (k b) -> p k b", k=2)
            nc.vector.tensor_scalar(
                out=o, in0=wsel, scalar1=float(g), scalar2=None, op0=Alu.mult
            )

        # ---------- one-hot row/col masks ----------
        # m4[s][t, (dir, b, i)] = 1 iff floor(x_dirb * G) == i, via
        # (iota <= x) * (iota > x-1); x = coord*G is exact (G power of two)
        m4 = []
        for s, g in enumerate(GS):
            io = consts[0:64, IOTA_OFF[s] : IOTA_OFF[s] + 4 * g].rearrange(
                "p (k g) -> p k g", k=4
            )
            xb = x4[:, 4 * s : 4 * s + 4][:, :, None].broadcast_to([64, 4, g])
            xm1b = x4m1[:, 4 * s : 4 * s + 4][:, :, None].broadcast_to([64, 4, g])
            at = sb.tile([64, 4 * g], f32, tag=f"onehA{s}")
            bt = sb.tile([64, 4 * g], f32, tag=f"onehB{s}")
            mt = sb.tile([64, 4 * g], f32, tag=f"m4_{s}")
            atr = at[:].rearrange("p (k g) -> p k g", k=4)
            btr = bt[:].rearrange("p (k g) -> p k g", k=4)
            nc.vector.tensor_tensor(out=atr, in0=io, in1=xb, op=Alu.is_le)
            nc.vector.tensor_tensor(out=btr, in0=io, in1=xm1b, op=Alu.is_gt)
            nc.vector.tensor_tensor(out=mt[:], in0=at[:], in1=bt[:], op=Alu.mult)
            m4.append(mt)

        # ---------- IoU / best-anchor (free layout (s, a, b) = [64, 18]) ----------
        def r3(t):  # [64,18] -> [64,3,3,2]
            return t[:].rearrange("p (s a b) -> p s a b", s=3, a=3)

        twth_r = twth[:].rearrange("p (s d b) -> p s d b", s=3, d=2)
        anc_r = ancb.rearrange("p (s a d) -> p s a d", s=3, a=3)
        tw_b = twth_r[:, :, 0, :][:, :, None, :].broadcast_to([64, 3, 3, 2])
        th_b = twth_r[:, :, 1, :][:, :, None, :].broadcast_to([64, 3, 3, 2])
        aw_b = anc_r[:, :, :, 0][:, :, :, None].broadcast_to([64, 3, 3, 2])
        ah_b = anc_r[:, :, :, 1][:, :, :, None].broadcast_to([64, 3, 3, 2])

        m1 = sb.tile([64, 18], f32, tag="m1")
        m2 = sb.tile([64, 18], f32, tag="m2")
        inter = sb.tile([64, 18], f32, tag="inter")
        nc.vector.tensor_tensor(out=r3(m1), in0=tw_b, in1=aw_b, op=Alu.min)
        nc.vector.tensor_tensor(out=r3(m2), in0=th_b, in1=ah_b, op=Alu.min)
        nc.vector.tensor_tensor(out=inter[:], in0=m1[:], in1=m2[:], op=Alu.mult)

        areat = sb.tile([64, 6], f32, tag="areat")  # (s, b) = tw*th
        nc.vector.tensor_tensor(
            out=areat[:].rearrange("p (s b) -> p s b", s=3),
            in0=twth_r[:, :, 0, :],
            in1=twth_r[:, :, 1, :],
            op=Alu.mult,
        )
        areaa = sb.tile([64, 9], f32, tag="areaa")  # (s, a) = aw*ah
        nc.vector.tensor_tensor(
            out=areaa[:].rearrange("p (s a) -> p s a", s=3),
            in0=anc_r[:, :, :, 0],
            in1=anc_r[:, :, :, 1],
            op=Alu.mult,
        )

        union = sb.tile([64, 18], f32, tag="union")
        areaa_b = (
            areaa[:]
            .rearrange("p (s a) -> p s a", s=3)[:, :, :, None]
            .broadcast_to([64, 3, 3, 2])
        )
        areat_b = (
            areat[:]
            .rearrange("p (s b) -> p s b", s=3)[:, :, None, :]
            .broadcast_to([64, 3, 3, 2])
        )
        nc.vector.tensor_tensor(out=r3(union), in0=areaa_b, in1=areat_b, op=Alu.add)
        nc.vector.tensor_tensor(
            out=union[:], in0=union[:], in1=inter[:], op=Alu.subtract
        )

        # iou > 0.5  <=>  2*inter > union   (division-free)
        cmp2 = sb.tile([64, 18], f32, tag="cmp2")
        nc.vector.scalar_tensor_tensor(
            out=cmp2[:],
            in0=inter[:],
            scalar=2.0,
            in1=union[:],
            op0=Alu.mult,
            op1=Alu.is_gt,
        )

        # argmax over anchors via cross products (iou_a >= iou_b <=>
        # inter_a*union_b >= inter_b*union_a); first-wins tie-breaking
        inter_r = r3(inter)
        union_r = r3(union)

        def pairprod(name, ia, ib):
            t = sb.tile([64, 6], f32, tag=name)
            nc.vector.tensor_tensor(
                out=t[:].rearrange("p (s b) -> p s b", s=3),
                in0=inter_r[:, :, ia, :],
                in1=union_r[:, :, ib, :],
                op=Alu.mult,
            )
            return t

        p01 = pairprod("p01", 0, 1)
        p10 = pairprod("p10", 1, 0)
        p02 = pairprod("p02", 0, 2)
        p20 = pairprod("p20", 2, 0)
        p12 = pairprod("p12", 1, 2)
        p21 = pairprod("p21", 2, 1)
        ge01 = sb.tile([64, 6], f32, tag="ge01")
        ge02 = sb.tile([64, 6], f32, tag="ge02")
        ge12 = sb.tile([64, 6], f32, tag="ge12")
        nc.vector.tensor_tensor(out=ge01[:], in0=p01[:], in1=p10[:], op=Alu.is_ge)
        nc.vector.tensor_tensor(out=ge02[:], in0=p02[:], in1=p20[:], op=Alu.is_ge)
        nc.vector.tensor_tensor(out=ge12[:], in0=p12[:], in1=p21[:], op=Alu.is_ge)

        oht = sb.tile([64, 18], f32, tag="oht")
        oht_r = r3(oht)
        # oh0 = ge01 & ge02
        nc.vector.tensor_tensor(
            out=oht_r[:, :, 0, :],
            in0=ge01[:].rearrange("p (s b) -> p s b", s=3),
            in1=ge02[:].rearrange("p (s b) -> p s b", s=3),
            op=Alu.mult,
        )
        # oh1 = (1 - ge01) & ge12
        n01 = sb.tile([64, 6], f32, tag="n01")
        nc.vector.tensor_scalar(
            out=n01[:],
            in0=ge01[:],
            scalar1=-1.0,
            scalar2=1.0,
            op0=Alu.mult,
            op1=Alu.add,
        )
        nc.vector.tensor_tensor(
            out=oht_r[:, :, 1, :],
            in0=n01[:].rearrange("p (s b) -> p s b", s=3),
            in1=ge12[:].rearrange("p (s b) -> p s b", s=3),
            op=Alu.mult,
        )
        # oh2 = 1 - oh0 - oh1  (oh0, oh1 mutually exclusive)
        s01 = sb.tile([64, 6], f32, tag="s01")
        nc.vector.tensor_tensor(
            out=s01[:].rearrange("p (s b) -> p s b", s=3),
            in0=oht_r[:, :, 0, :],
            in1=oht_r[:, :, 1, :],
            op=Alu.add,
        )
        nc.vector.tensor_scalar(
            out=oht_r[:, :, 2, :],
            in0=s01[:].rearrange("p (s b) -> p s b", s=3),
            scalar1=-1.0,
            scalar2=1.0,
            op0=Alu.mult,
            op1=Alu.add,
        )

        # w4 = onehot(best anchor) & (iou > 0.5)
        w4 = sb.tile([64, 18], f32, tag="w4")
        nc.vector.tensor_tensor(out=w4[:], in0=oht[:], in1=cmp2[:], op=Alu.mult)

        # ---------- mja = one-hot(j) per (b, a), weighted; one tile/scale ----------
        # cols (b, a, j): mja[t, (b,a,j)] = onehotJ[t, b, j] * w4[t, s, a, b]
        mja = []
        for s, g in enumerate(GS):
            t = sb.tile([64, BL * 3 * g], f32, tag=f"mja{s}")
            mj_b = (
                m4[s][:, 2 * g : 4 * g]
                .rearrange("p (b g) -> p b g", b=2)[:, :, None, :]
                .broadcast_to([64, 2, 3, g])
            )
            wv = (
                r3(w4)[:, s, :, :]
                .rearrange("p a b -> p b a")[:, :, :, None]
                .broadcast_to([64, 2, 3, g])
            )
            nc.vector.tensor_tensor(
                out=t[:].rearrange("p (b a g) -> p b a g", b=2, a=3),
                in0=mj_b,
                in1=wv,
                op=Alu.mult,
            )
            mja.append(t)

        # ---------- per-scale: gt matmuls, BCE ----------
        gts = []
        for s, g in enumerate(GS):
            P, NCH = PPART[s], NCHUNK[s]
            pt = ps.tile([P, NCH * g], f32, tag=f"gt{s}")
            for cidx in range(NCH):
                b = (cidx * P) // (A * g)  # batch of this chunk
                nc.tensor.matmul(
                    pt[:, cidx * g : (cidx + 1) * g],
                    mja[s][:, cidx * P : (cidx + 1) * P],
                    m4[s][:, b * g : (b + 1) * g],
                    start=True,
                    stop=True,
                )
            gts.append(pt)

        # ---------- per-chunk BCE, fired as each chunk's gather lands ----------
        # Per-chunk processing keeps every consumer at ONE sem wait (its
        # chunk's DMA lane), and the per-chunk [P,2] partial sums stream out
        # to DRAM as they finish — the host does the final cross-partition /
        # cross-chunk reduction (a "psum of per-shard sums").
        k = 0
        for s, g in enumerate(GS):
            P, NCH = PPART[s], NCHUNK[s]
            for cidx in range(NCH):
              # pin each chunk's BCE late in the virtual schedule so the
              # Tile scheduler cannot interleave it into the target-math /
              # matmul section of the in-order engine streams (the BCE
              # waits on the slow gathers; anything scheduled after it
              # would stall)
              with tc.tile_wait_until(1.0 + 0.01 * k):
                pr = pred[s][:, cidx * g : (cidx + 1) * g]
                l1 = sb.tile([P, g], f32, tag=f"l1_{k}")
                l2 = sb.tile([P, g], f32, tag=f"l2_{k}")
                dd = sb.tile([P, g], f32, tag=f"dd{k}")
                gg = sb.tile([P, g], f32, tag=f"gg{k}")
                ak = sb.tile([P, 2], f32, tag=f"ak{k}")
                nc.scalar.activation(
                    out=l1[:],
                    in_=pr,
                    func=Act.Ln,
                    bias=consts[0:P, ONE_OFF : ONE_OFF + 1],
                    scale=-1.0,
                )
                nc.scalar.activation(
                    out=l2[:],
                    in_=pr,
                    func=Act.Ln,
                    bias=consts[0:P, ZERO_OFF : ZERO_OFF + 1],
                )
                # binarize gt counts (sole op waiting on PE)
                gtb = sb.tile([P, g], f32, tag=f"gtb{k}")
                nc.vector.tensor_scalar(
                    out=gtb[:],
                    in0=gts[s][:, cidx * g : (cidx + 1) * g],
                    scalar1=0.5,
                    scalar2=None,
                    op0=Alu.is_ge,
                )
                nc.vector.tensor_tensor(
                    out=dd[:], in0=l1[:], in1=l2[:], op=Alu.subtract
                )
                # ak[:,1] = sum(L1), computed on DVE so that ak has a
                # single writer engine (the out-DMA can encode one wait)
                l1c = sb.tile([P, g], f32, tag=f"l1c{k}")
                nc.vector.tensor_scalar(
                    out=l1c[:],
                    in0=l1[:],
                    scalar1=0.0,
                    scalar2=0.0,
                    op0=Alu.add,
                    op1=Alu.add,
                    accum_out=ak[:, 1:2],
                )
                # gg = gtb * (L1 - L2); ak[:,0] = sum(gg)
                nc.vector.scalar_tensor_tensor(
                    out=gg[:],
                    in0=dd[:],
                    scalar=0.0,
                    in1=gtb[:],
                    op0=Alu.bypass,
                    op1=Alu.mult,
                    accum_out=ak[:, 0:1],
                )
                # stream this chunk's partials out via the idle gpsimd
                # SWDGE so the scalar engine only runs the two Lns per
                # chunk (8 x ~550ns of DMA-issue slices would otherwise
                # serialize into the scalar stream at the tail)
                nc.gpsimd.dma_start(
                    out=d_part[0:P, 2 * k : 2 * k + 2], in_=ak[:]
                )
              k += 1

    _fixup_tail_drain(nc, mybir)
    _release_pe_early(nc, mybir)
    _BUILT = nc
    return nc


def _release_pe_early(nc, mybir):
    """Drop the PE (Tensor) engine from the two exit-barrier rounds so it
    proceeds straight to the NEFF epilogue (its ~3.5us of per-semaphore
    clears then overlap the gather phase instead of serializing after it).
    Safe because the last gtb on DVE waits for the FINAL PE semaphore
    count, so DVE's own barrier arrival already implies every PE update
    retired; PE's cleared range (low sem ids) is only used by the
    entry/exit barriers themselves."""
    for blk in nc.m.functions[0].blocks:
        if not blk.name.endswith("_end"):
            continue
        insts = blk.instructions
        keep = []
        removed = 0
        for inst in insts:
            tn = type(inst).__name__
            eng = getattr(inst, "engine", None)
            if (
                tn in ("InstDrain", "InstEventSemaphore")
                and eng is not None
                and eng.name == "PE"
            ):
                removed += 1
                continue
            keep.append(inst)
        assert removed == 4, f"expected 4 PE exit instructions, got {removed}"
        blk.instructions[:] = keep
        # the Pool-side barrier now gathers 3 engines instead of 4
        fixed = 0
        for inst in blk.instructions:
            if type(inst).__name__ != "InstEventSemaphore":
                continue
            si = inst.sync_info
            for w in si.on_wait if si else []:
                if w.wait_value == 4:
                    w.wait_value = 3
                    fixed += 1
            for u in si.on_update if si else []:
                if u.update_value == 4:
                    u.update_value = 3
                    fixed += 1
        assert fixed >= 4, f"expected >=4 barrier-count fixes, got {fixed}"


def _trim_epilogue(nc):
    """Tile's exit emits TWO all-engine drain+barrier rounds (one before the
    semaphore clear, one after).  The second round only orders the clear
    against a subsequent kernel *within the same execution*; across
    executions the runtime itself serializes, so drop round two.  The Pool
    semaphore-clear/dma-reset instructions stay (they run before Pool's
    round-one barrier release)."""
    for blk in nc.m.functions[0].blocks:
        if not blk.name.endswith("_end"):
            continue
        insts = blk.instructions
        # locate the second drain round: the second InstDrain on Activation
        seen_act = 0
        cut = None
        for i, inst in enumerate(insts):
            if (
                type(inst).__name__ == "InstDrain"
                and inst.engine is not None
                and inst.engine.name == "Activation"
            ):
                seen_act += 1
                if seen_act == 2:
                    cut = i
                    break
        if cut is not None:
            del insts[cut:]


def _merge_gather_sems(nc, mybir):
    """Each chunk DMA of a scale gets its own completion semaphore from the
    tile framework, but the ISA allows one sync wait per instruction and the
    per-scale Ln consumes all chunks of a scale at once.  Point every chunk
    DMA of a scale at the first chunk's semaphore (each still +16) and
    rewrite consumers to a single wait for the cumulative count.  Safe
    because every consumer of a scale tile needs ALL of its chunks."""
    import re

    fn = nc.m.functions[0]
    groups = {}  # dest tensor name -> [InstDMACopy...] in program order
    for blk in fn.blocks:
        for inst in blk.instructions:
            if type(inst).__name__ != "InstDMACopy":
                continue
            src = str(inst.ins[0]) if inst.ins else ""
            if not re.search(r"name='out[012]'", src):
                continue
            d = re.search(r"name='([^']+)'", str(inst.outs[0])).group(1)
            groups.setdefault(d, []).append(inst)
    remap = {}  # old sem id -> (merged id, cumulative wait value)
    for d, insts in groups.items():
        if len(insts) < 2:
            continue
        base = insts[0].sync_info.on_update[0].id
        base_name = insts[0].sync_info.on_update[0].ant_name
        total = 0
        olds = []
        for inst in insts:
            u = inst.sync_info.on_update[0]
            olds.append(u.id)
            u.id = base
            u.ant_name = base_name
            total += u.update_value
        for o in olds:
            remap[o] = (base, base_name, total)
    for blk in fn.blocks:
        for inst in blk.instructions:
            si = inst.sync_info
            if not si or not si.on_wait:
                continue
            if not any(w.id in remap for w in si.on_wait):
                continue
            keep = [w for w in si.on_wait if w.id not in remap]
            seen = set()
            for w in si.on_wait:
                if w.id not in remap:
                    continue
                base, base_name, total = remap[w.id]
                if base in seen:
                    continue
                seen.add(base)
                w.id = base
                w.ant_name = base_name
                w.wait_value = total
                keep.append(w)
            inst.sync_info = mybir.SyncInfo(
                on_wait=keep, on_update=list(si.on_update)
            )


def _fixup_tail_drain(nc, mybir):
    """The kernel-tail drains wait on every outstanding semaphore lane, but
    the ISA allows one sync wait per instruction and this walrus refuses to
    split them.  The lanes that MUST be gated at the tail are the ones no
    compute instruction observes: the SWDGE lanes carrying the 'partial'
    output DMAs (every gather lane is observed by its Ln).  Distribute
    those lanes, one wait per tail drain, across the multiple drains."""
    fn = nc.m.functions[0]
    # Merge every partial-writing DMA's completion update onto the FIRST
    # one's semaphore.  No compute instruction waits on these sems (the
    # partial tensor is only read by the host), so the only observer that
    # needs rewriting is the tail drain.
    base = base_name = None
    total = 0
    for blk in fn.blocks:
        for inst in blk.instructions:
            if type(inst).__name__ == "InstDMACopy":
                outs = inst.outs
                if outs and ("partial" in str(outs[0])):
                    u = inst.sync_info.on_update[0]
                    if base is None:
                        base, base_name = u.id, u.ant_name
                    u.id, u.ant_name = base, base_name
                    total += u.update_value
    assert base is not None, "no output DMA with sem update found"
    did = 0
    for blk in fn.blocks:
        for inst in blk.instructions:
            si = inst.sync_info
            if (
                type(inst).__name__ == "InstDrain"
                and si is not None
                and len(si.on_wait) > 1
            ):
                w = next(x for x in si.on_wait if x.id == base)
                w.ant_name = base_name
                w.wait_value = total
                inst.sync_info = mybir.SyncInfo(
                    on_wait=[w], on_update=list(si.on_update)
                )
                did += 1
    assert did >= 1, "no multi-wait tail drain found"


def _make_in_maps(out0, out1, out2, anchors0, anchors1, anchors2, targets):
    base = _const_base()
    anc_flat = np.concatenate(
        [np.asarray(a, np.float32).reshape(-1) for a in (anchors0, anchors1, anchors2)]
    )  # (s, a, d) = 18
    outs = (out0, out1, out2)
    in_maps = []
    for c in range(NCORES):
        sl = slice(c * BL, (c + 1) * BL)
        consts = base.copy()
        consts[:, ANC_OFF : ANC_OFF + 18] = anc_flat[None, :]
        # targets block: rows = t, cols = (b, k)
        tloc = np.asarray(targets[sl], np.float32)  # [BL, T, 5]
        consts[0:T, TGT_OFF : TGT_OFF + 10] = tloc.transpose(1, 0, 2).reshape(T, -1)
        m = {"consts": consts}
        for s in range(3):
            m[f"out{s}"] = np.ascontiguousarray(outs[s][sl])
        in_maps.append(m)
    return in_maps


def _reduce_partials(partials):
    """partials: list of [96, 16] per-core arrays (cols = (chunk, {gg, l1}),
    rows = partitions; only rows [0:PPART[s]] of a chunk are valid) ->
    scalar loss (float64 accum)."""
    tot = np.zeros(6, np.float64)
    for p in partials:
        p = np.asarray(p, np.float64)
        k = 0
        for s in range(3):
            P = PPART[s]
            for _ in range(NCHUNK[s]):
                tot[2 * s] += p[0:P, 2 * k].sum()
                tot[2 * s + 1] += p[0:P, 2 * k + 1].sum()
                k += 1
    loss = 0.0
    for s, g in enumerate(GS):
        denom = B * A * g * g
        loss += (tot[2 * s] - tot[2 * s + 1]) / denom
    return np.float32(loss)


def _run_hw(in_maps, trace=False):
    from concourse.bass_utils import run_bass_kernel_spmd

    nc = _build()
    br = run_bass_kernel_spmd(nc, in_maps, list(range(NCORES)), trace=trace)
    return br


def kernel(out0, out1, out2, anchors0, anchors1, anchors2, targets):
    in_maps = _make_in_maps(
        out0, out1, out2, anchors0, anchors1, anchors2, targets
    )
    br = _run_hw(in_maps, trace=False)
    partials = [r["partial"] for r in br.results]
    return np.asarray(_reduce_partials(partials), dtype=np.float32)


# revision 38
# speedup vs baseline: 1.0657x; 1.0657x over previous
"""Trainium2 Bass kernel for nn_ObjectLoss (YOLO-style objectness BCE loss).

Reference semantics (per scale s with grid G):
    pred = out_s[..., 4]                            # objectness channel
    per-target best anchor by IoU of (w,h) boxes; cells (b, a*, ty*G, tx*G)
    with iou > 0.5 get gt=1 (idempotent scatter)
    loss_s = mean(-(gt*log(p) + (1-gt)*log1p(-p)))
    loss = sum over 3 scales

Strategy (8 cores, data-parallel over batch, 2 batches/core):
  - Only channel 4 of 85 is ever needed: gather it with strided DMA
    (1/85th of the bytes).  The gather is SDMA-packet-bound: HWDGE emits
    one packet per 4B descriptor (~11.5 ns/pkt/engine), while SWDGE
    (gpsimd) aggregates ~57 descriptors per packet (~3.5 ns/desc of
    SDMA engine time).  So the big scale rides SWDGE and a ~6k-desc
    slice rides the sync HWDGE ring to use leftover SDMA capacity
    (SWDGE generation, at ~0.41 ns/desc, is the serial bottleneck).
  - One gather DMA per scale; rows (b, a, h) are packed into column
    chunks of P partitions so each scale lands in a single SBUF tile:
    s0 [96, 4*64], s1 [96, 2*32], s2 [48, 2*16].
  - gt grid built on-device without scatter: one-hot(row) x one-hot(col)
    outer products accumulated over targets == one matmul per 128-row
    chunk into a per-scale PSUM tile.
  - BCE = -sum(L1) + sum(gt*(L1-L2)) with L1=ln(1-p), L2=ln(p), computed
    per scale with ACT-engine Ln + fused accumulators; per-core partial
    sums are reduced on host (psum of per-shard sums).

Hardware note: each compute instruction can encode only ONE semaphore
wait, so the program is shaped to give every instruction at most one
unobserved cross-engine dependency: all small inputs ride in a single
"consts" DMA, each engine touches it early, and psum-consuming ops are
split so they wait only on the PE semaphore.
"""

import os
import sys

import numpy as np

for _p in ("/opt/trn_rl_repo", "/root/.axon_site/_ro/trn_rl_repo"):
    if os.path.isdir(_p) and _p not in sys.path:
        sys.path.insert(0, _p)
        break

GS = (64, 32, 16)  # grid size per scale (H == W)
B, A, T, C = 16, 3, 64, 85
NCORES = 8
BL = B // NCORES  # batches per core
OBJ = 4  # objectness channel

# per-scale packing: rows (b a h) -> (chunk, partition); chunks never
# cross batch boundaries (matmul out base partition must be 0)
PPART = (96, 96, 48)  # partitions per chunk
NCHUNK = (4, 2, 2)  # chunks per scale (NCHUNK*PPART == BL*A*g)

# consts layout [128, NCONST]: per-scale iota repeated 4x, anchors
# (replicated across partitions), targets re-laid-out as [t, (b k)],
# a ones column and a zeros column (activation bias operands).
IOTA_OFF = []
_off = 0
for _g in GS:
    IOTA_OFF.append(_off)
    _off += 4 * _g
ANC_OFF = _off          # 18 cols: (s, a, d)
TGT_OFF = _off + 18     # 10 cols: (b, k), rows = t
ONE_OFF = TGT_OFF + 10  # 1.0
ZERO_OFF = ONE_OFF + 1  # 0.0
NCONST = ZERO_OFF + 1

_CONST_BASE = None


def _const_base():
    global _CONST_BASE
    if _CONST_BASE is None:
        c = np.zeros((128, NCONST), np.float32)
        for s, g in enumerate(GS):
            c[:, IOTA_OFF[s] : IOTA_OFF[s] + 4 * g] = np.tile(
                np.arange(g, dtype=np.float32), 4
            )[None, :]
        c[:, ONE_OFF] = 1.0
        _CONST_BASE = c
    return _CONST_BASE


_BUILT = None


def _build():
    """Build the SPMD bass program (same program on all 8 cores)."""
    global _BUILT
    if _BUILT is not None:
        return _BUILT

    from contextlib import ExitStack

    import concourse.bass as bass
    import concourse.tile as tile
    from concourse import mybir

    f32 = mybir.dt.float32
    Alu = mybir.AluOpType
    Act = mybir.ActivationFunctionType

    nc = bass.Bass()
    d_outs = [
        nc.declare_dram_parameter(f"out{s}", [BL, A, g, g, C], f32, isOutput=False)
        for s, g in enumerate(GS)
    ]
    d_const = nc.declare_dram_parameter("consts", [128, NCONST], f32, isOutput=False)
    # per-chunk, per-partition partial sums: cols (chunk, {gg, l1})
    d_part = nc.declare_dram_parameter("partial", [96, 16], f32, isOutput=True)

    with tile.TileContext(nc) as tc, ExitStack() as ctx:
        sb = ctx.enter_context(tc.tile_pool(name="sb", bufs=1))
        ps = ctx.enter_context(tc.tile_pool(name="ps", bufs=1, space="PSUM"))

        # ---------- gathers first: no deps, start the SDMA engines ----------
        # objectness channel of scale s as [chunk, partition, w]; the
        # dest AP iterates (c, p, w) to match the row-major src rows.
        pred = []
        for s, g in enumerate(GS):
            P, NCH = PPART[s], NCHUNK[s]
            t = sb.tile([P, NCH * g], f32, tag=f"pred{s}")
            pred.append(t)

        def gather(eng, s):
            g = GS[s]
            P, NCH = PPART[s], NCHUNK[s]
            rows = d_outs[s][:].rearrange("b a h w c -> (b a h) w c")
            with nc.allow_non_contiguous_dma("objectness channel gather"):
                for cidx in range(NCH):
                    src = rows[cidx * P : (cidx + 1) * P, :, OBJ]
                    dst = pred[s][:, cidx * g : (cidx + 1) * g]
                    eng.dma_start(out=dst, in_=src)

        # consts first (everything on the vector engine needs it), then the
        # gathers, biggest scale first so its BCE overlaps later gathers.
        # All on sync HWDGE: the gather is bound at ~11.5 ns per 4B
        # descriptor per SDMA engine (HBM-read issue cost); neither SWDGE
        # aggregation nor a second DGE ring changes that, and SWDGE adds
        # generation stalls on top.
        # consts on the scalar (ACT) HWDGE ring: off the sync ring so the
        # gathers own it entirely, off gpsimd so the 8 ak out-DMAs don't
        # exceed the 8 SWDGE sem lanes (a 9th forces a lane-reuse guard
        # wait that walrus can't encode alongside the data dep)
        consts = sb.tile([128, NCONST], f32, tag="consts")
        nc.scalar.dma_start(out=consts[:], in_=d_const[:])
        gather(nc.sync, 0)     # 24576 descs
        gather(nc.sync, 1)     # 6144 descs
        gather(nc.sync, 2)     # 1536 descs

        # ACT warm-up touch of consts so later activations never need a
        # consts wait (one sem wait max per instruction).
        warm = sb.tile([1, 1], f32, tag="warm")
        nc.scalar.copy(warm[:], consts[0:1, 0:1])

        ancb = consts[0:64, ANC_OFF : ANC_OFF + 18]  # (s, a, d)
        tgt = consts[0:64, TGT_OFF : TGT_OFF + 10]  # rows=t, cols=(b, k)

        # ---------- per-target math (all [64, *] tiles; partition = t) ----------
        tgt_kb = tgt.rearrange("p (b k) -> p k b", b=BL)  # [64, 5, BL]
        xsel = tgt_kb[:, 1:3, :]  # (tx, ty) per b
        wsel = tgt_kb[:, 3:5, :]  # (tw, th) per b

        x4 = sb.tile([64, 12], f32, tag="x4")  # (s, dir, b): x*G
        x4m1 = sb.tile([64, 12], f32, tag="x4m1")  # x*G - 1
        twth = sb.tile([64, 12], f32, tag="twth")  # (s, d, b): box wh in grid units
        for s, g in enumerate(GS):
            o = x4[:, 4 * s : 4 * s + 4].rearrange("p (k b) -> p k b", k=2)
            nc.vector.tensor_scalar(
                out=o, in0=xsel, scalar1=float(g), scalar2=None, op0=Alu.mult
            )
            o = x4m1[:, 4 * s : 4 * s + 4].rearrange("p (k b) -> p k b", k=2)
            nc.vector.tensor_scalar(
                out=o,
                in0=xsel,
                scalar1=float(g),
                scalar2=1.0,
                op0=Alu.mult,
                op1=Alu.subtract,
            )
            o = twth[:, 4 * s : 4 * s + 4].rearrange("p (k b) -> p k b", k=2)
            nc.vector.tensor_scalar(
                out=o, in0=wsel, scalar1=float(g), scalar2=None, op0=Alu.mult
            )

        # ---------- one-hot row/col masks ----------
        # m4[s][t, (dir, b, i)] = 1 iff floor(x_dirb * G) == i, via
        # (iota <= x) * (iota > x-1); x = coord*G is exact (G power of two)
        m4 = []
        for s, g in enumerate(GS):
            io = consts[0:64, IOTA_OFF[s] : IOTA_OFF[s] + 4 * g].rearrange(
                "p (k g) -> p k g", k=4
            )
            xb = x4[:, 4 * s : 4 * s + 4][:, :, None].broadcast_to([64, 4, g])
            xm1b = x4m1[:, 4 * s : 4 * s + 4][:, :, None].broadcast_to([64, 4, g])
            at = sb.tile([64, 4 * g], f32, tag=f"onehA{s}")
            bt = sb.tile([64, 4 * g], f32, tag=f"onehB{s}")
            mt = sb.tile([64, 4 * g], f32, tag=f"m4_{s}")
            atr = at[:].rearrange("p (k g) -> p k g", k=4)
            btr = bt[:].rearrange("p (k g) -> p k g", k=4)
            nc.vector.tensor_tensor(out=atr, in0=io, in1=xb, op=Alu.is_le)
            nc.vector.tensor_tensor(out=btr, in0=io, in1=xm1b, op=Alu.is_gt)
            nc.vector.tensor_tensor(out=mt[:], in0=at[:], in1=bt[:], op=Alu.mult)
            m4.append(mt)

        # ---------- IoU / best-anchor (free layout (s, a, b) = [64, 18]) ----------
        def r3(t):  # [64,18] -> [64,3,3,2]
            return t[:].rearrange("p (s a b) -> p s a b", s=3, a=3)

        twth_r = twth[:].rearrange("p (s d b) -> p s d b", s=3, d=2)
        anc_r = ancb.rearrange("p (s a d) -> p s a d", s=3, a=3)
        tw_b = twth_r[:, :, 0, :][:, :, None, :].broadcast_to([64, 3, 3, 2])
        th_b = twth_r[:, :, 1, :][:, :, None, :].broadcast_to([64, 3, 3, 2])
        aw_b = anc_r[:, :, :, 0][:, :, :, None].broadcast_to([64, 3, 3, 2])
        ah_b = anc_r[:, :, :, 1][:, :, :, None].broadcast_to([64, 3, 3, 2])

        m1 = sb.tile([64, 18], f32, tag="m1")
        m2 = sb.tile([64, 18], f32, tag="m2")
        inter = sb.tile([64, 18], f32, tag="inter")
        nc.vector.tensor_tensor(out=r3(m1), in0=tw_b, in1=aw_b, op=Alu.min)
        nc.vector.tensor_tensor(out=r3(m2), in0=th_b, in1=ah_b, op=Alu.min)
        nc.vector.tensor_tensor(out=inter[:], in0=m1[:], in1=m2[:], op=Alu.mult)

        areat = sb.tile([64, 6], f32, tag="areat")  # (s, b) = tw*th
        nc.vector.tensor_tensor(
            out=areat[:].rearrange("p (s b) -> p s b", s=3),
            in0=twth_r[:, :, 0, :],
            in1=twth_r[:, :, 1, :],
            op=Alu.mult,
        )
        areaa = sb.tile([64, 9], f32, tag="areaa")  # (s, a) = aw*ah
        nc.vector.tensor_tensor(
            out=areaa[:].rearrange("p (s a) -> p s a", s=3),
            in0=anc_r[:, :, :, 0],
            in1=anc_r[:, :, :, 1],
            op=Alu.mult,
        )

        union = sb.tile([64, 18], f32, tag="union")
        areaa_b = (
            areaa[:]
            .rearrange("p (s a) -> p s a", s=3)[:, :, :, None]
            .broadcast_to([64, 3, 3, 2])
        )
        areat_b = (
            areat[:]
            .rearrange("p (s b) -> p s b", s=3)[:, :, None, :]
            .broadcast_to([64, 3, 3, 2])
        )
        nc.vector.tensor_tensor(out=r3(union), in0=areaa_b, in1=areat_b, op=Alu.add)
        nc.vector.tensor_tensor(
            out=union[:], in0=union[:], in1=inter[:], op=Alu.subtract
        )

        # iou > 0.5  <=>  2*inter > union   (division-free)
        cmp2 = sb.tile([64, 18], f32, tag="cmp2")
        nc.vector.scalar_tensor_tensor(
            out=cmp2[:],
            in0=inter[:],
            scalar=2.0,
            in1=union[:],
            op0=Alu.mult,
            op1=Alu.is_gt,
        )

        # argmax over anchors via cross products (iou_a >= iou_b <=>
        # inter_a*union_b >= inter_b*union_a); first-wins tie-breaking
        inter_r = r3(inter)
        union_r = r3(union)

        def pairprod(name, ia, ib):
            t = sb.tile([64, 6], f32, tag=name)
            nc.vector.tensor_tensor(
                out=t[:].rearrange("p (s b) -> p s b", s=3),
                in0=inter_r[:, :, ia, :],
                in1=union_r[:, :, ib, :],
                op=Alu.mult,
            )
            return t

        p01 = pairprod("p01", 0, 1)
        p10 = pairprod("p10", 1, 0)
        p02 = pairprod("p02", 0, 2)
        p20 = pairprod("p20", 2, 0)
        p12 = pairprod("p12", 1, 2)
        p21 = pairprod("p21", 2, 1)
        ge01 = sb.tile([64, 6], f32, tag="ge01")
        ge02 = sb.tile([64, 6], f32, tag="ge02")
        ge12 = sb.tile([64, 6], f32, tag="ge12")
        nc.vector.tensor_tensor(out=ge01[:], in0=p01[:], in1=p10[:], op=Alu.is_ge)
        nc.vector.tensor_tensor(out=ge02[:], in0=p02[:], in1=p20[:], op=Alu.is_ge)
        nc.vector.tensor_tensor(out=ge12[:], in0=p12[:], in1=p21[:], op=Alu.is_ge)

        oht = sb.tile([64, 18], f32, tag="oht")
        oht_r = r3(oht)
        # oh0 = ge01 & ge02
        nc.vector.tensor_tensor(
            out=oht_r[:, :, 0, :],
            in0=ge01[:].rearrange("p (s b) -> p s b", s=3),
            in1=ge02[:].rearrange("p (s b) -> p s b", s=3),
            op=Alu.mult,
        )
        # oh1 = (1 - ge01) & ge12
        n01 = sb.tile([64, 6], f32, tag="n01")
        nc.vector.tensor_scalar(
            out=n01[:],
            in0=ge01[:],
            scalar1=-1.0,
            scalar2=1.0,
            op0=Alu.mult,
            op1=Alu.add,
        )
        nc.vector.tensor_tensor(
            out=oht_r[:, :, 1, :],
            in0=n01[:].rearrange("p (s b) -> p s b", s=3),
            in1=ge12[:].rearrange("p (s b) -> p s b", s=3),
            op=Alu.mult,
        )
        # oh2 = 1 - oh0 - oh1  (oh0, oh1 mutually exclusive)
        s01 = sb.tile([64, 6], f32, tag="s01")
        nc.vector.tensor_tensor(
            out=s01[:].rearrange("p (s b) -> p s b", s=3),
            in0=oht_r[:, :, 0, :],
            in1=oht_r[:, :, 1, :],
            op=Alu.add,
        )
        nc.vector.tensor_scalar(
            out=oht_r[:, :, 2, :],
            in0=s01[:].rearrange("p (s b) -> p s b", s=3),
            scalar1=-1.0,
            scalar2=1.0,
            op0=Alu.mult,
            op1=Alu.add,
        )

        # w4 = onehot(best anchor) & (iou > 0.5)
        w4 = sb.tile([64, 18], f32, tag="w4")
        nc.vector.tensor_tensor(out=w4[:], in0=oht[:], in1=cmp2[:], op=Alu.mult)

        # ---------- mja = one-hot(j) per (b, a), weighted; one tile/scale ----------
        # cols (b, a, j): mja[t, (b,a,j)] = onehotJ[t, b, j] * w4[t, s, a, b]
        mja = []
        for s, g in enumerate(GS):
            t = sb.tile([64, BL * 3 * g], f32, tag=f"mja{s}")
            mj_b = (
                m4[s][:, 2 * g : 4 * g]
                .rearrange("p (b g) -> p b g", b=2)[:, :, None, :]
                .broadcast_to([64, 2, 3, g])
            )
            wv = (
                r3(w4)[:, s, :, :]
                .rearrange("p a b -> p b a")[:, :, :, None]
                .broadcast_to([64, 2, 3, g])
            )
            nc.vector.tensor_tensor(
                out=t[:].rearrange("p (b a g) -> p b a g", b=2, a=3),
                in0=mj_b,
                in1=wv,
                op=Alu.mult,
            )
            mja.append(t)

        # ---------- per-scale: gt matmuls, BCE ----------
        gts = []
        for s, g in enumerate(GS):
            P, NCH = PPART[s], NCHUNK[s]
            pt = ps.tile([P, NCH * g], f32, tag=f"gt{s}")
            for cidx in range(NCH):
                b = (cidx * P) // (A * g)  # batch of this chunk
                nc.tensor.matmul(
                    pt[:, cidx * g : (cidx + 1) * g],
                    mja[s][:, cidx * P : (cidx + 1) * P],
                    m4[s][:, b * g : (b + 1) * g],
                    start=True,
                    stop=True,
                )
            gts.append(pt)

        # ---------- per-chunk BCE, fired as each chunk's gather lands ----------
        # Per-chunk processing keeps every consumer at ONE sem wait (its
        # chunk's DMA lane), and the per-chunk [P,2] partial sums stream out
        # to DRAM as they finish — the host does the final cross-partition /
        # cross-chunk reduction (a "psum of per-shard sums").
        k = 0
        for s, g in enumerate(GS):
            P, NCH = PPART[s], NCHUNK[s]
            for cidx in range(NCH):
              # pin each chunk's BCE late in the virtual schedule so the
              # Tile scheduler cannot interleave it into the target-math /
              # matmul section of the in-order engine streams (the BCE
              # waits on the slow gathers; anything scheduled after it
              # would stall)
              with tc.tile_wait_until(1.0 + 0.01 * k):
                pr = pred[s][:, cidx * g : (cidx + 1) * g]
                l1 = sb.tile([P, g], f32, tag=f"l1_{k}")
                l2 = sb.tile([P, g], f32, tag=f"l2_{k}")
                dd = sb.tile([P, g], f32, tag=f"dd{k}")
                gg = sb.tile([P, g], f32, tag=f"gg{k}")
                ak = sb.tile([P, 2], f32, tag=f"ak{k}")
                nc.scalar.activation(
                    out=l1[:],
                    in_=pr,
                    func=Act.Ln,
                    bias=consts[0:P, ONE_OFF : ONE_OFF + 1],
                    scale=-1.0,
                )
                nc.scalar.activation(
                    out=l2[:],
                    in_=pr,
                    func=Act.Ln,
                    bias=consts[0:P, ZERO_OFF : ZERO_OFF + 1],
                )
                # binarize gt counts (sole op waiting on PE)
                gtb = sb.tile([P, g], f32, tag=f"gtb{k}")
                nc.vector.tensor_scalar(
                    out=gtb[:],
                    in0=gts[s][:, cidx * g : (cidx + 1) * g],
                    scalar1=0.5,
                    scalar2=None,
                    op0=Alu.is_ge,
                )
                # ak[:,1] = sum(L1), computed on DVE so that ak has a
                # single writer engine (the out-DMA can encode one wait);
                # issued before dd so it overlaps l2's activation
                l1c = sb.tile([P, g], f32, tag=f"l1c{k}")
                nc.vector.tensor_scalar(
                    out=l1c[:],
                    in0=l1[:],
                    scalar1=0.0,
                    scalar2=0.0,
                    op0=Alu.add,
                    op1=Alu.add,
                    accum_out=ak[:, 1:2],
                )
                nc.vector.tensor_tensor(
                    out=dd[:], in0=l1[:], in1=l2[:], op=Alu.subtract
                )
                # gg = gtb * (L1 - L2); ak[:,0] = sum(gg)
                nc.vector.scalar_tensor_tensor(
                    out=gg[:],
                    in0=dd[:],
                    scalar=0.0,
                    in1=gtb[:],
                    op0=Alu.bypass,
                    op1=Alu.mult,
                    accum_out=ak[:, 0:1],
                )
                # stream this chunk's partials out via the idle gpsimd
                # SWDGE so the scalar engine only runs the two Lns per
                # chunk (8 x ~550ns of DMA-issue slices would otherwise
                # serialize into the scalar stream at the tail)
                nc.gpsimd.dma_start(
                    out=d_part[0:P, 2 * k : 2 * k + 2], in_=ak[:]
                )
              k += 1

    _fixup_tail_drain(nc, mybir)
    _hoist_gathers(nc)
    _BUILT = nc
    return nc


def _hoist_gathers(nc):
    """Move the wait-free gather/consts DMA issues to the front of their
    engines' instruction streams, ahead of the bass-init all-engine
    barrier events.  The barrier orders SBUF const-tile memsets against
    compute engines; the DMA issues touch neither, and hoisting them
    starts the 23us SDMA drain ~1us earlier."""
    for blk in nc.m.functions[0].blocks:
        insts = blk.instructions
        by_eng = {}
        for inst in insts:
            eng = getattr(inst, "engine", None)
            if eng is None:
                continue
            by_eng.setdefault(eng.name, []).append(inst)
        hoist = set()
        for name in ("SP", "Activation"):
            stream = by_eng.get(name, [])
            for inst in stream:
                if (
                    type(inst).__name__ == "InstDMACopy"
                    and not (inst.sync_info and inst.sync_info.on_wait)
                ):
                    hoist.add(id(inst))
        if not hoist:
            continue
        moved = [i for i in insts if id(i) in hoist]
        rest = [i for i in insts if id(i) not in hoist]
        blk.instructions[:] = moved + rest


def _release_pe_early(nc, mybir):
    """Drop the PE (Tensor) engine from the two exit-barrier rounds so it
    proceeds straight to the NEFF epilogue (its ~3.5us of per-semaphore
    clears then overlap the gather phase instead of serializing after it).
    Safe because the last gtb on DVE waits for the FINAL PE semaphore
    count, so DVE's own barrier arrival already implies every PE update
    retired; PE's cleared range (low sem ids) is only used by the
    entry/exit barriers themselves."""
    for blk in nc.m.functions[0].blocks:
        if not blk.name.endswith("_end"):
            continue
        insts = blk.instructions
        keep = []
        removed = 0
        for inst in insts:
            tn = type(inst).__name__
            eng = getattr(inst, "engine", None)
            if (
                tn in ("InstDrain", "InstEventSemaphore")
                and eng is not None
                and eng.name == "PE"
            ):
                removed += 1
                continue
            keep.append(inst)
        assert removed == 4, f"expected 4 PE exit instructions, got {removed}"
        blk.instructions[:] = keep
        # the Pool-side barrier now gathers 3 engines instead of 4
        fixed = 0
        for inst in blk.instructions:
            if type(inst).__name__ != "InstEventSemaphore":
                continue
            si = inst.sync_info
            for w in si.on_wait if si else []:
                if w.wait_value == 4:
                    w.wait_value = 3
                    fixed += 1
            for u in si.on_update if si else []:
                if u.update_value == 4:
                    u.update_value = 3
                    fixed += 1
        assert fixed >= 4, f"expected >=4 barrier-count fixes, got {fixed}"


def _trim_epilogue(nc):
    """Tile's exit emits TWO all-engine drain+barrier rounds (one before the
    semaphore clear, one after).  The second round only orders the clear
    against a subsequent kernel *within the same execution*; across
    executions the runtime itself serializes, so drop round two.  The Pool
    semaphore-clear/dma-reset instructions stay (they run before Pool's
    round-one barrier release)."""
    for blk in nc.m.functions[0].blocks:
        if not blk.name.endswith("_end"):
            continue
        insts = blk.instructions
        # locate the second drain round: the second InstDrain on Activation
        seen_act = 0
        cut = None
        for i, inst in enumerate(insts):
            if (
                type(inst).__name__ == "InstDrain"
                and inst.engine is not None
                and inst.engine.name == "Activation"
            ):
                seen_act += 1
                if seen_act == 2:
                    cut = i
                    break
        if cut is not None:
            del insts[cut:]


def _merge_gather_sems(nc, mybir):
    """Each chunk DMA of a scale gets its own completion semaphore from the
    tile framework, but the ISA allows one sync wait per instruction and the
    per-scale Ln consumes all chunks of a scale at once.  Point every chunk
    DMA of a scale at the first chunk's semaphore (each still +16) and
    rewrite consumers to a single wait for the cumulative count.  Safe
    because every consumer of a scale tile needs ALL of its chunks."""
    import re

    fn = nc.m.functions[0]
    groups = {}  # dest tensor name -> [InstDMACopy...] in program order
    for blk in fn.blocks:
        for inst in blk.instructions:
            if type(inst).__name__ != "InstDMACopy":
                continue
            src = str(inst.ins[0]) if inst.ins else ""
            if not re.search(r"name='out[012]'", src):
                continue
            d = re.search(r"name='([^']+)'", str(inst.outs[0])).group(1)
            groups.setdefault(d, []).append(inst)
    remap = {}  # old sem id -> (merged id, cumulative wait value)
    for d, insts in groups.items():
        if len(insts) < 2:
            continue
        base = insts[0].sync_info.on_update[0].id
        base_name = insts[0].sync_info.on_update[0].ant_name
        total = 0
        olds = []
        for inst in insts:
            u = inst.sync_info.on_update[0]
            olds.append(u.id)
            u.id = base
            u.ant_name = base_name
            total += u.update_value
        for o in olds:
            remap[o] = (base, base_name, total)
    for blk in fn.blocks:
        for inst in blk.instructions:
            si = inst.sync_info
            if not si or not si.on_wait:
                continue
            if not any(w.id in remap for w in si.on_wait):
                continue
            keep = [w for w in si.on_wait if w.id not in remap]
            seen = set()
            for w in si.on_wait:
                if w.id not in remap:
                    continue
                base, base_name, total = remap[w.id]
                if base in seen:
                    continue
                seen.add(base)
                w.id = base
                w.ant_name = base_name
                w.wait_value = total
                keep.append(w)
            inst.sync_info = mybir.SyncInfo(
                on_wait=keep, on_update=list(si.on_update)
            )


def _fixup_tail_drain(nc, mybir):
    """The kernel-tail drains wait on every outstanding semaphore lane, but
    the ISA allows one sync wait per instruction and this walrus refuses to
    split them.  The lanes that MUST be gated at the tail are the ones no
    compute instruction observes: the SWDGE lanes carrying the 'partial'
    output DMAs (every gather lane is observed by its Ln).  Distribute
    those lanes, one wait per tail drain, across the multiple drains."""
    fn = nc.m.functions[0]
    # Merge every partial-writing DMA's completion update onto the FIRST
    # one's semaphore.  No compute instruction waits on these sems (the
    # partial tensor is only read by the host), so the only observer that
    # needs rewriting is the tail drain.
    base = base_name = None
    total = 0
    for blk in fn.blocks:
        for inst in blk.instructions:
            if type(inst).__name__ == "InstDMACopy":
                outs = inst.outs
                if outs and ("partial" in str(outs[0])):
                    u = inst.sync_info.on_update[0]
                    if base is None:
                        base, base_name = u.id, u.ant_name
                    u.id, u.ant_name = base, base_name
                    total += u.update_value
    assert base is not None, "no output DMA with sem update found"
    did = 0
    for blk in fn.blocks:
        for inst in blk.instructions:
            si = inst.sync_info
            if (
                type(inst).__name__ == "InstDrain"
                and si is not None
                and len(si.on_wait) > 1
            ):
                w = next(x for x in si.on_wait if x.id == base)
                w.ant_name = base_name
                w.wait_value = total
                inst.sync_info = mybir.SyncInfo(
                    on_wait=[w], on_update=list(si.on_update)
                )
                did += 1
    assert did >= 1, "no multi-wait tail drain found"


def _make_in_maps(out0, out1, out2, anchors0, anchors1, anchors2, targets):
    base = _const_base()
    anc_flat = np.concatenate(
        [np.asarray(a, np.float32).reshape(-1) for a in (anchors0, anchors1, anchors2)]
    )  # (s, a, d) = 18
    outs = (out0, out1, out2)
    in_maps = []
    for c in range(NCORES):
        sl = slice(c * BL, (c + 1) * BL)
        consts = base.copy()
        consts[:, ANC_OFF : ANC_OFF + 18] = anc_flat[None, :]
        # targets block: rows = t, cols = (b, k)
        tloc = np.asarray(targets[sl], np.float32)  # [BL, T, 5]
        consts[0:T, TGT_OFF : TGT_OFF + 10] = tloc.transpose(1, 0, 2).reshape(T, -1)
        m = {"consts": consts}
        for s in range(3):
            m[f"out{s}"] = np.ascontiguousarray(outs[s][sl])
        in_maps.append(m)
    return in_maps


def _reduce_partials(partials):
    """partials: list of [96, 16] per-core arrays (cols = (chunk, {gg, l1}),
    rows = partitions; only rows [0:PPART[s]] of a chunk are valid) ->
    scalar loss (float64 accum)."""
    tot = np.zeros(6, np.float64)
    for p in partials:
        p = np.asarray(p, np.float64)
        k = 0
        for s in range(3):
            P = PPART[s]
            for _ in range(NCHUNK[s]):
                tot[2 * s] += p[0:P, 2 * k].sum()
                tot[2 * s + 1] += p[0:P, 2 * k + 1].sum()
                k += 1
    loss = 0.0
    for s, g in enumerate(GS):
        denom = B * A * g * g
        loss += (tot[2 * s] - tot[2 * s + 1]) / denom
    return np.float32(loss)


def _run_hw(in_maps, trace=False):
    from concourse.bass_utils import run_bass_kernel_spmd

    nc = _build()
    br = run_bass_kernel_spmd(nc, in_maps, list(range(NCORES)), trace=trace)
    return br


def kernel(out0, out1, out2, anchors0, anchors1, anchors2, targets):
    in_maps = _make_in_maps(
        out0, out1, out2, anchors0, anchors1, anchors2, targets
    )
    br = _run_hw(in_maps, trace=False)
    partials = [r["partial"] for r in br.results]
    return np.asarray(_reduce_partials(partials), dtype=np.float32)


# revision 42
# speedup vs baseline: 1.0895x; 1.0224x over previous
"""Trainium2 Bass kernel for nn_ObjectLoss (YOLO-style objectness BCE loss).

Reference semantics (per scale s with grid G):
    pred = out_s[..., 4]                            # objectness channel
    per-target best anchor by IoU of (w,h) boxes; cells (b, a*, ty*G, tx*G)
    with iou > 0.5 get gt=1 (idempotent scatter)
    loss_s = mean(-(gt*log(p) + (1-gt)*log1p(-p)))
    loss = sum over 3 scales

Strategy (8 cores, data-parallel over batch, 2 batches/core):
  - Only channel 4 of 85 is ever needed: gather it with strided DMA
    (1/85th of the bytes).  The gather is SDMA-packet-bound: HWDGE emits
    one packet per 4B descriptor (~11.5 ns/pkt/engine), while SWDGE
    (gpsimd) aggregates ~57 descriptors per packet (~3.5 ns/desc of
    SDMA engine time).  So the big scale rides SWDGE and a ~6k-desc
    slice rides the sync HWDGE ring to use leftover SDMA capacity
    (SWDGE generation, at ~0.41 ns/desc, is the serial bottleneck).
  - One gather DMA per scale; rows (b, a, h) are packed into column
    chunks of P partitions so each scale lands in a single SBUF tile:
    s0 [96, 4*64], s1 [96, 2*32], s2 [48, 2*16].
  - gt grid built on-device without scatter: one-hot(row) x one-hot(col)
    outer products accumulated over targets == one matmul per 128-row
    chunk into a per-scale PSUM tile.
  - BCE = -sum(L1) + sum(gt*(L1-L2)) with L1=ln(1-p), L2=ln(p), computed
    per scale with ACT-engine Ln + fused accumulators; per-core partial
    sums are reduced on host (psum of per-shard sums).

Hardware note: each compute instruction can encode only ONE semaphore
wait, so the program is shaped to give every instruction at most one
unobserved cross-engine dependency: all small inputs ride in a single
"consts" DMA, each engine touches it early, and psum-consuming ops are
split so they wait only on the PE semaphore.
"""

import os
import sys

import numpy as np

for _p in ("/opt/trn_rl_repo", "/root/.axon_site/_ro/trn_rl_repo"):
    if os.path.isdir(_p) and _p not in sys.path:
        sys.path.insert(0, _p)
        break

GS = (64, 32, 16)  # grid size per scale (H == W)
B, A, T, C = 16, 3, 64, 85
NCORES = 8
BL = B // NCORES  # batches per core
OBJ = 4  # objectness channel

# per-scale packing: rows (b a h) -> (chunk, partition); chunks never
# cross batch boundaries (matmul out base partition must be 0)
PPART = (96, 96, 48)  # partitions per chunk
NCHUNK = (4, 2, 2)  # chunks per scale (NCHUNK*PPART == BL*A*g)

# consts layout [128, NCONST]: anchors (replicated across partitions),
# targets re-laid-out as [t, (b k)], a ones column and a zeros column
# (activation bias operands).  The grid iotas are generated on-device.
ANC_OFF = 0             # 18 cols: (s, a, d)
TGT_OFF = 18            # 10 cols: (b, k), rows = t
ONE_OFF = TGT_OFF + 10  # 1.0
ZERO_OFF = ONE_OFF + 1  # 0.0
NCONST = ZERO_OFF + 1

_CONST_BASE = None


def _const_base():
    global _CONST_BASE
    if _CONST_BASE is None:
        c = np.zeros((128, NCONST), np.float32)
        c[:, ONE_OFF] = 1.0
        _CONST_BASE = c
    return _CONST_BASE


_BUILT = None


def _build():
    """Build the SPMD bass program (same program on all 8 cores)."""
    global _BUILT
    if _BUILT is not None:
        return _BUILT

    from contextlib import ExitStack

    import concourse.bass as bass
    import concourse.tile as tile
    from concourse import mybir

    f32 = mybir.dt.float32
    Alu = mybir.AluOpType
    Act = mybir.ActivationFunctionType

    nc = bass.Bass()
    d_outs = [
        nc.declare_dram_parameter(f"out{s}", [BL, A, g, g, C], f32, isOutput=False)
        for s, g in enumerate(GS)
    ]
    d_const = nc.declare_dram_parameter("consts", [128, NCONST], f32, isOutput=False)
    # per-chunk, per-partition partial sums: cols (chunk, {gg, l1})
    d_part = nc.declare_dram_parameter("partial", [96, 16], f32, isOutput=True)

    with tile.TileContext(nc) as tc, ExitStack() as ctx:
        sb = ctx.enter_context(tc.tile_pool(name="sb", bufs=1))
        ps = ctx.enter_context(tc.tile_pool(name="ps", bufs=1, space="PSUM"))

        # ---------- gathers first: no deps, start the SDMA engines ----------
        # objectness channel of scale s as [chunk, partition, w]; the
        # dest AP iterates (c, p, w) to match the row-major src rows.
        pred = []
        for s, g in enumerate(GS):
            P, NCH = PPART[s], NCHUNK[s]
            t = sb.tile([P, NCH * g], f32, tag=f"pred{s}")
            pred.append(t)

        def gather(eng, s):
            g = GS[s]
            P, NCH = PPART[s], NCHUNK[s]
            rows = d_outs[s][:].rearrange("b a h w c -> (b a h) w c")
            with nc.allow_non_contiguous_dma("objectness channel gather"):
                for cidx in range(NCH):
                    src = rows[cidx * P : (cidx + 1) * P, :, OBJ]
                    dst = pred[s][:, cidx * g : (cidx + 1) * g]
                    eng.dma_start(out=dst, in_=src)

        # consts first (everything on the vector engine needs it), then the
        # gathers, biggest scale first so its BCE overlaps later gathers.
        # All on sync HWDGE: the gather is bound at ~11.5 ns per 4B
        # descriptor per SDMA engine (HBM-read issue cost); neither SWDGE
        # aggregation nor a second DGE ring changes that, and SWDGE adds
        # generation stalls on top.
        # consts on the scalar (ACT) HWDGE ring: off the sync ring so the
        # gathers own it entirely, off gpsimd so the 8 ak out-DMAs don't
        # exceed the 8 SWDGE sem lanes (a 9th forces a lane-reuse guard
        # wait that walrus can't encode alongside the data dep)
        consts = sb.tile([128, NCONST], f32, tag="consts")
        nc.scalar.dma_start(out=consts[:], in_=d_const[:])
        gather(nc.sync, 0)     # 24576 descs
        gather(nc.sync, 1)     # 6144 descs
        gather(nc.sync, 2)     # 1536 descs

        # ACT warm-up touch of consts so later activations never need a
        # consts wait (one sem wait max per instruction).
        warm = sb.tile([1, 1], f32, tag="warm")
        nc.scalar.copy(warm[:], consts[0:1, 0:1])

        ancb = consts[0:64, ANC_OFF : ANC_OFF + 18]  # (s, a, d)
        tgt = consts[0:64, TGT_OFF : TGT_OFF + 10]  # rows=t, cols=(b, k)

        # ---------- per-target math (all [64, *] tiles; partition = t) ----------
        tgt_kb = tgt.rearrange("p (b k) -> p k b", b=BL)  # [64, 5, BL]
        xsel = tgt_kb[:, 1:3, :]  # (tx, ty) per b
        wsel = tgt_kb[:, 3:5, :]  # (tw, th) per b

        x4 = sb.tile([64, 12], f32, tag="x4")  # (s, dir, b): x*G
        x4m1 = sb.tile([64, 12], f32, tag="x4m1")  # x*G - 1
        twth = sb.tile([64, 12], f32, tag="twth")  # (s, d, b): box wh in grid units
        for s, g in enumerate(GS):
            o = x4[:, 4 * s : 4 * s + 4].rearrange("p (k b) -> p k b", k=2)
            nc.vector.tensor_scalar(
                out=o, in0=xsel, scalar1=float(g), scalar2=None, op0=Alu.mult
            )
            o = x4m1[:, 4 * s : 4 * s + 4].rearrange("p (k b) -> p k b", k=2)
            nc.vector.tensor_scalar(
                out=o,
                in0=xsel,
                scalar1=float(g),
                scalar2=1.0,
                op0=Alu.mult,
                op1=Alu.subtract,
            )
            o = twth[:, 4 * s : 4 * s + 4].rearrange("p (k b) -> p k b", k=2)
            nc.vector.tensor_scalar(
                out=o, in0=wsel, scalar1=float(g), scalar2=None, op0=Alu.mult
            )

        # ---------- one-hot row/col masks ----------
        # m4[s][t, (dir, b, i)] = 1 iff floor(x_dirb * G) == i, via
        # (iota <= x) * (iota > x-1); x = coord*G is exact (G power of two)
        iot = []
        for s, g in enumerate(GS):
            io_t = sb.tile([64, 4 * g], f32, tag=f"iota{s}")
            nc.gpsimd.iota(
                io_t[:].rearrange("p (k g) -> p k g", k=4),
                pattern=[[0, 4], [1, g]],
                base=0,
                channel_multiplier=0,
                allow_small_or_imprecise_dtypes=True,
            )
            iot.append(io_t)
        # vector observer touch of the last iota absorbs the Pool sem so
        # the one-hot compares keep a single wait each
        warmv = sb.tile([1, 1], f32, tag="warmv")
        nc.vector.tensor_copy(warmv[:], iot[2][0:1, 0:1])

        m4 = []
        for s, g in enumerate(GS):
            io = iot[s][:].rearrange("p (k g) -> p k g", k=4)
            xb = x4[:, 4 * s : 4 * s + 4][:, :, None].broadcast_to([64, 4, g])
            xm1b = x4m1[:, 4 * s : 4 * s + 4][:, :, None].broadcast_to([64, 4, g])
            at = sb.tile([64, 4 * g], f32, tag=f"onehA{s}")
            bt = sb.tile([64, 4 * g], f32, tag=f"onehB{s}")
            mt = sb.tile([64, 4 * g], f32, tag=f"m4_{s}")
            atr = at[:].rearrange("p (k g) -> p k g", k=4)
            btr = bt[:].rearrange("p (k g) -> p k g", k=4)
            nc.vector.tensor_tensor(out=atr, in0=io, in1=xb, op=Alu.is_le)
            nc.vector.tensor_tensor(out=btr, in0=io, in1=xm1b, op=Alu.is_gt)
            nc.vector.tensor_tensor(out=mt[:], in0=at[:], in1=bt[:], op=Alu.mult)
            m4.append(mt)

        # ---------- IoU / best-anchor (free layout (s, a, b) = [64, 18]) ----------
        def r3(t):  # [64,18] -> [64,3,3,2]
            return t[:].rearrange("p (s a b) -> p s a b", s=3, a=3)

        twth_r = twth[:].rearrange("p (s d b) -> p s d b", s=3, d=2)
        anc_r = ancb.rearrange("p (s a d) -> p s a d", s=3, a=3)
        tw_b = twth_r[:, :, 0, :][:, :, None, :].broadcast_to([64, 3, 3, 2])
        th_b = twth_r[:, :, 1, :][:, :, None, :].broadcast_to([64, 3, 3, 2])
        aw_b = anc_r[:, :, :, 0][:, :, :, None].broadcast_to([64, 3, 3, 2])
        ah_b = anc_r[:, :, :, 1][:, :, :, None].broadcast_to([64, 3, 3, 2])

        m1 = sb.tile([64, 18], f32, tag="m1")
        m2 = sb.tile([64, 18], f32, tag="m2")
        inter = sb.tile([64, 18], f32, tag="inter")
        nc.vector.tensor_tensor(out=r3(m1), in0=tw_b, in1=aw_b, op=Alu.min)
        nc.vector.tensor_tensor(out=r3(m2), in0=th_b, in1=ah_b, op=Alu.min)
        nc.vector.tensor_tensor(out=inter[:], in0=m1[:], in1=m2[:], op=Alu.mult)

        areat = sb.tile([64, 6], f32, tag="areat")  # (s, b) = tw*th
        nc.vector.tensor_tensor(
            out=areat[:].rearrange("p (s b) -> p s b", s=3),
            in0=twth_r[:, :, 0, :],
            in1=twth_r[:, :, 1, :],
            op=Alu.mult,
        )
        areaa = sb.tile([64, 9], f32, tag="areaa")  # (s, a) = aw*ah
        nc.vector.tensor_tensor(
            out=areaa[:].rearrange("p (s a) -> p s a", s=3),
            in0=anc_r[:, :, :, 0],
            in1=anc_r[:, :, :, 1],
            op=Alu.mult,
        )

        union = sb.tile([64, 18], f32, tag="union")
        areaa_b = (
            areaa[:]
            .rearrange("p (s a) -> p s a", s=3)[:, :, :, None]
            .broadcast_to([64, 3, 3, 2])
        )
        areat_b = (
            areat[:]
            .rearrange("p (s b) -> p s b", s=3)[:, :, None, :]
            .broadcast_to([64, 3, 3, 2])
        )
        nc.vector.tensor_tensor(out=r3(union), in0=areaa_b, in1=areat_b, op=Alu.add)
        nc.vector.tensor_tensor(
            out=union[:], in0=union[:], in1=inter[:], op=Alu.subtract
        )

        # iou > 0.5  <=>  2*inter > union   (division-free)
        cmp2 = sb.tile([64, 18], f32, tag="cmp2")
        nc.vector.scalar_tensor_tensor(
            out=cmp2[:],
            in0=inter[:],
            scalar=2.0,
            in1=union[:],
            op0=Alu.mult,
            op1=Alu.is_gt,
        )

        # argmax over anchors via cross products (iou_a >= iou_b <=>
        # inter_a*union_b >= inter_b*union_a); first-wins tie-breaking
        inter_r = r3(inter)
        union_r = r3(union)

        def pairprod(name, ia, ib):
            t = sb.tile([64, 6], f32, tag=name)
            nc.vector.tensor_tensor(
                out=t[:].rearrange("p (s b) -> p s b", s=3),
                in0=inter_r[:, :, ia, :],
                in1=union_r[:, :, ib, :],
                op=Alu.mult,
            )
            return t

        p01 = pairprod("p01", 0, 1)
        p10 = pairprod("p10", 1, 0)
        p02 = pairprod("p02", 0, 2)
        p20 = pairprod("p20", 2, 0)
        p12 = pairprod("p12", 1, 2)
        p21 = pairprod("p21", 2, 1)
        ge01 = sb.tile([64, 6], f32, tag="ge01")
        ge02 = sb.tile([64, 6], f32, tag="ge02")
        ge12 = sb.tile([64, 6], f32, tag="ge12")
        nc.vector.tensor_tensor(out=ge01[:], in0=p01[:], in1=p10[:], op=Alu.is_ge)
        nc.vector.tensor_tensor(out=ge02[:], in0=p02[:], in1=p20[:], op=Alu.is_ge)
        nc.vector.tensor_tensor(out=ge12[:], in0=p12[:], in1=p21[:], op=Alu.is_ge)

        oht = sb.tile([64, 18], f32, tag="oht")
        oht_r = r3(oht)
        # oh0 = ge01 & ge02
        nc.vector.tensor_tensor(
            out=oht_r[:, :, 0, :],
            in0=ge01[:].rearrange("p (s b) -> p s b", s=3),
            in1=ge02[:].rearrange("p (s b) -> p s b", s=3),
            op=Alu.mult,
        )
        # oh1 = (1 - ge01) & ge12
        n01 = sb.tile([64, 6], f32, tag="n01")
        nc.vector.tensor_scalar(
            out=n01[:],
            in0=ge01[:],
            scalar1=-1.0,
            scalar2=1.0,
            op0=Alu.mult,
            op1=Alu.add,
        )
        nc.vector.tensor_tensor(
            out=oht_r[:, :, 1, :],
            in0=n01[:].rearrange("p (s b) -> p s b", s=3),
            in1=ge12[:].rearrange("p (s b) -> p s b", s=3),
            op=Alu.mult,
        )
        # oh2 = 1 - oh0 - oh1  (oh0, oh1 mutually exclusive)
        s01 = sb.tile([64, 6], f32, tag="s01")
        nc.vector.tensor_tensor(
            out=s01[:].rearrange("p (s b) -> p s b", s=3),
            in0=oht_r[:, :, 0, :],
            in1=oht_r[:, :, 1, :],
            op=Alu.add,
        )
        nc.vector.tensor_scalar(
            out=oht_r[:, :, 2, :],
            in0=s01[:].rearrange("p (s b) -> p s b", s=3),
            scalar1=-1.0,
            scalar2=1.0,
            op0=Alu.mult,
            op1=Alu.add,
        )

        # w4 = onehot(best anchor) & (iou > 0.5)
        w4 = sb.tile([64, 18], f32, tag="w4")
        nc.vector.tensor_tensor(out=w4[:], in0=oht[:], in1=cmp2[:], op=Alu.mult)

        # ---------- mja = one-hot(j) per (b, a), weighted; one tile/scale ----------
        # cols (b, a, j): mja[t, (b,a,j)] = onehotJ[t, b, j] * w4[t, s, a, b]
        mja = []
        for s, g in enumerate(GS):
            t = sb.tile([64, BL * 3 * g], f32, tag=f"mja{s}")
            mj_b = (
                m4[s][:, 2 * g : 4 * g]
                .rearrange("p (b g) -> p b g", b=2)[:, :, None, :]
                .broadcast_to([64, 2, 3, g])
            )
            wv = (
                r3(w4)[:, s, :, :]
                .rearrange("p a b -> p b a")[:, :, :, None]
                .broadcast_to([64, 2, 3, g])
            )
            nc.vector.tensor_tensor(
                out=t[:].rearrange("p (b a g) -> p b a g", b=2, a=3),
                in0=mj_b,
                in1=wv,
                op=Alu.mult,
            )
            mja.append(t)

        # ---------- per-scale: gt matmuls, BCE ----------
        gts = []
        for s, g in enumerate(GS):
            P, NCH = PPART[s], NCHUNK[s]
            pt = ps.tile([P, NCH * g], f32, tag=f"gt{s}")
            for cidx in range(NCH):
                b = (cidx * P) // (A * g)  # batch of this chunk
                nc.tensor.matmul(
                    pt[:, cidx * g : (cidx + 1) * g],
                    mja[s][:, cidx * P : (cidx + 1) * P],
                    m4[s][:, b * g : (b + 1) * g],
                    start=True,
                    stop=True,
                )
            gts.append(pt)

        # ---------- per-chunk BCE, fired as each chunk's gather lands ----------
        # Per-chunk processing keeps every consumer at ONE sem wait (its
        # chunk's DMA lane), and the per-chunk [P,2] partial sums stream out
        # to DRAM as they finish — the host does the final cross-partition /
        # cross-chunk reduction (a "psum of per-shard sums").
        k = 0
        for s, g in enumerate(GS):
            P, NCH = PPART[s], NCHUNK[s]
            for cidx in range(NCH):
              # pin each chunk's BCE late in the virtual schedule so the
              # Tile scheduler cannot interleave it into the target-math /
              # matmul section of the in-order engine streams (the BCE
              # waits on the slow gathers; anything scheduled after it
              # would stall)
              with tc.tile_wait_until(1.0 + 0.01 * k):
                pr = pred[s][:, cidx * g : (cidx + 1) * g]
                l1 = sb.tile([P, g], f32, tag=f"l1_{k}")
                l2 = sb.tile([P, g], f32, tag=f"l2_{k}")
                dd = sb.tile([P, g], f32, tag=f"dd{k}")
                gg = sb.tile([P, g], f32, tag=f"gg{k}")
                ak = sb.tile([P, 2], f32, tag=f"ak{k}")
                nc.scalar.activation(
                    out=l1[:],
                    in_=pr,
                    func=Act.Ln,
                    bias=consts[0:P, ONE_OFF : ONE_OFF + 1],
                    scale=-1.0,
                )
                nc.scalar.activation(
                    out=l2[:],
                    in_=pr,
                    func=Act.Ln,
                    bias=consts[0:P, ZERO_OFF : ZERO_OFF + 1],
                )
                # binarize gt counts (sole op waiting on PE)
                gtb = sb.tile([P, g], f32, tag=f"gtb{k}")
                nc.vector.tensor_scalar(
                    out=gtb[:],
                    in0=gts[s][:, cidx * g : (cidx + 1) * g],
                    scalar1=0.5,
                    scalar2=None,
                    op0=Alu.is_ge,
                )
                # ak[:,1] = sum(L1), computed on DVE so that ak has a
                # single writer engine (the out-DMA can encode one wait);
                # issued before dd so it overlaps l2's activation
                l1c = sb.tile([P, g], f32, tag=f"l1c{k}")
                nc.vector.tensor_scalar(
                    out=l1c[:],
                    in0=l1[:],
                    scalar1=0.0,
                    scalar2=0.0,
                    op0=Alu.add,
                    op1=Alu.add,
                    accum_out=ak[:, 1:2],
                )
                nc.vector.tensor_tensor(
                    out=dd[:], in0=l1[:], in1=l2[:], op=Alu.subtract
                )
                # gg = gtb * (L1 - L2); ak[:,0] = sum(gg)
                nc.vector.scalar_tensor_tensor(
                    out=gg[:],
                    in0=dd[:],
                    scalar=0.0,
                    in1=gtb[:],
                    op0=Alu.bypass,
                    op1=Alu.mult,
                    accum_out=ak[:, 0:1],
                )
                # stream this chunk's partials out via the idle gpsimd
                # SWDGE so the scalar engine only runs the two Lns per
                # chunk (8 x ~550ns of DMA-issue slices would otherwise
                # serialize into the scalar stream at the tail)
                nc.gpsimd.dma_start(
                    out=d_part[0:P, 2 * k : 2 * k + 2], in_=ak[:]
                )
              k += 1

    _fixup_tail_drain(nc, mybir)
    _hoist_gathers(nc)
    _BUILT = nc
    return nc


def _hoist_gathers(nc):
    """Move the wait-free gather/consts DMA issues to the front of their
    engines' instruction streams, ahead of the bass-init all-engine
    barrier events.  The barrier orders SBUF const-tile memsets against
    compute engines; the DMA issues touch neither, and hoisting them
    starts the 23us SDMA drain ~1us earlier."""
    for blk in nc.m.functions[0].blocks:
        insts = blk.instructions
        by_eng = {}
        for inst in insts:
            eng = getattr(inst, "engine", None)
            if eng is None:
                continue
            by_eng.setdefault(eng.name, []).append(inst)
        hoist = set()
        for name in ("SP", "Activation"):
            stream = by_eng.get(name, [])
            for inst in stream:
                if (
                    type(inst).__name__ == "InstDMACopy"
                    and not (inst.sync_info and inst.sync_info.on_wait)
                ):
                    hoist.add(id(inst))
        if not hoist:
            continue
        moved = [i for i in insts if id(i) in hoist]
        rest = [i for i in insts if id(i) not in hoist]
        blk.instructions[:] = moved + rest


def _release_pe_early(nc, mybir):
    """Drop the PE (Tensor) engine from the two exit-barrier rounds so it
    proceeds straight to the NEFF epilogue (its ~3.5us of per-semaphore
    clears then overlap the gather phase instead of serializing after it).
    Safe because the last gtb on DVE waits for the FINAL PE semaphore
    count, so DVE's own barrier arrival already implies every PE update
    retired; PE's cleared range (low sem ids) is only used by the
    entry/exit barriers themselves."""
    for blk in nc.m.functions[0].blocks:
        if not blk.name.endswith("_end"):
            continue
        insts = blk.instructions
        keep = []
        removed = 0
        for inst in insts:
            tn = type(inst).__name__
            eng = getattr(inst, "engine", None)
            if (
                tn in ("InstDrain", "InstEventSemaphore")
                and eng is not None
                and eng.name == "PE"
            ):
                removed += 1
                continue
            keep.append(inst)
        assert removed == 4, f"expected 4 PE exit instructions, got {removed}"
        blk.instructions[:] = keep
        # the Pool-side barrier now gathers 3 engines instead of 4
        fixed = 0
        for inst in blk.instructions:
            if type(inst).__name__ != "InstEventSemaphore":
                continue
            si = inst.sync_info
            for w in si.on_wait if si else []:
                if w.wait_value == 4:
                    w.wait_value = 3
                    fixed += 1
            for u in si.on_update if si else []:
                if u.update_value == 4:
                    u.update_value = 3
                    fixed += 1
        assert fixed >= 4, f"expected >=4 barrier-count fixes, got {fixed}"


def _trim_epilogue(nc):
    """Tile's exit emits TWO all-engine drain+barrier rounds (one before the
    semaphore clear, one after).  The second round only orders the clear
    against a subsequent kernel *within the same execution*; across
    executions the runtime itself serializes, so drop round two.  The Pool
    semaphore-clear/dma-reset instructions stay (they run before Pool's
    round-one barrier release)."""
    for blk in nc.m.functions[0].blocks:
        if not blk.name.endswith("_end"):
            continue
        insts = blk.instructions
        # locate the second drain round: the second InstDrain on Activation
        seen_act = 0
        cut = None
        for i, inst in enumerate(insts):
            if (
                type(inst).__name__ == "InstDrain"
                and inst.engine is not None
                and inst.engine.name == "Activation"
            ):
                seen_act += 1
                if seen_act == 2:
                    cut = i
                    break
        if cut is not None:
            del insts[cut:]


def _merge_gather_sems(nc, mybir):
    """Each chunk DMA of a scale gets its own completion semaphore from the
    tile framework, but the ISA allows one sync wait per instruction and the
    per-scale Ln consumes all chunks of a scale at once.  Point every chunk
    DMA of a scale at the first chunk's semaphore (each still +16) and
    rewrite consumers to a single wait for the cumulative count.  Safe
    because every consumer of a scale tile needs ALL of its chunks."""
    import re

    fn = nc.m.functions[0]
    groups = {}  # dest tensor name -> [InstDMACopy...] in program order
    for blk in fn.blocks:
        for inst in blk.instructions:
            if type(inst).__name__ != "InstDMACopy":
                continue
            src = str(inst.ins[0]) if inst.ins else ""
            if not re.search(r"name='out[012]'", src):
                continue
            d = re.search(r"name='([^']+)'", str(inst.outs[0])).group(1)
            groups.setdefault(d, []).append(inst)
    remap = {}  # old sem id -> (merged id, cumulative wait value)
    for d, insts in groups.items():
        if len(insts) < 2:
            continue
        base = insts[0].sync_info.on_update[0].id
        base_name = insts[0].sync_info.on_update[0].ant_name
        total = 0
        olds = []
        for inst in insts:
            u = inst.sync_info.on_update[0]
            olds.append(u.id)
            u.id = base
            u.ant_name = base_name
            total += u.update_value
        for o in olds:
            remap[o] = (base, base_name, total)
    for blk in fn.blocks:
        for inst in blk.instructions:
            si = inst.sync_info
            if not si or not si.on_wait:
                continue
            if not any(w.id in remap for w in si.on_wait):
                continue
            keep = [w for w in si.on_wait if w.id not in remap]
            seen = set()
            for w in si.on_wait:
                if w.id not in remap:
                    continue
                base, base_name, total = remap[w.id]
                if base in seen:
                    continue
                seen.add(base)
                w.id = base
                w.ant_name = base_name
                w.wait_value = total
                keep.append(w)
            inst.sync_info = mybir.SyncInfo(
                on_wait=keep, on_update=list(si.on_update)
            )


def _fixup_tail_drain(nc, mybir):
    """The kernel-tail drains wait on every outstanding semaphore lane, but
    the ISA allows one sync wait per instruction and this walrus refuses to
    split them.  The lanes that MUST be gated at the tail are the ones no
    compute instruction observes: the SWDGE lanes carrying the 'partial'
    output DMAs (every gather lane is observed by its Ln).  Distribute
    those lanes, one wait per tail drain, across the multiple drains."""
    fn = nc.m.functions[0]
    # Merge every partial-writing DMA's completion update onto the FIRST
    # one's semaphore.  No compute instruction waits on these sems (the
    # partial tensor is only read by the host), so the only observer that
    # needs rewriting is the tail drain.
    base = base_name = None
    total = 0
    for blk in fn.blocks:
        for inst in blk.instructions:
            if type(inst).__name__ == "InstDMACopy":
                outs = inst.outs
                if outs and ("partial" in str(outs[0])):
                    u = inst.sync_info.on_update[0]
                    if base is None:
                        base, base_name = u.id, u.ant_name
                    u.id, u.ant_name = base, base_name
                    total += u.update_value
    assert base is not None, "no output DMA with sem update found"
    did = 0
    for blk in fn.blocks:
        for inst in blk.instructions:
            si = inst.sync_info
            if (
                type(inst).__name__ == "InstDrain"
                and si is not None
                and len(si.on_wait) > 1
            ):
                w = next(x for x in si.on_wait if x.id == base)
                w.ant_name = base_name
                w.wait_value = total
                inst.sync_info = mybir.SyncInfo(
                    on_wait=[w], on_update=list(si.on_update)
                )
                did += 1
    assert did >= 1, "no multi-wait tail drain found"


def _make_in_maps(out0, out1, out2, anchors0, anchors1, anchors2, targets):
    base = _const_base()
    anc_flat = np.concatenate(
        [np.asarray(a, np.float32).reshape(-1) for a in (anchors0, anchors1, anchors2)]
    )  # (s, a, d) = 18
    outs = (out0, out1, out2)
    in_maps = []
    for c in range(NCORES):
        sl = slice(c * BL, (c + 1) * BL)
        consts = base.copy()
        consts[:, ANC_OFF : ANC_OFF + 18] = anc_flat[None, :]
        # targets block: rows = t, cols = (b, k)
        tloc = np.asarray(targets[sl], np.float32)  # [BL, T, 5]
        consts[0:T, TGT_OFF : TGT_OFF + 10] = tloc.transpose(1, 0, 2).reshape(T, -1)
        m = {"consts": consts}
        for s in range(3):
            m[f"out{s}"] = np.ascontiguousarray(outs[s][sl])
        in_maps.append(m)
    return in_maps


def _reduce_partials(partials):
    """partials: list of [96, 16] per-core arrays (cols = (chunk, {gg, l1}),
    rows = partitions; only rows [0:PPART[s]] of a chunk are valid) ->
    scalar loss (float64 accum)."""
    tot = np.zeros(6, np.float64)
    for p in partials:
        p = np.asarray(p, np.float64)
        k = 0
        for s in range(3):
            P = PPART[s]
            for _ in range(NCHUNK[s]):
                tot[2 * s] += p[0:P, 2 * k].sum()
                tot[2 * s + 1] += p[0:P, 2 * k + 1].sum()
                k += 1
    loss = 0.0
    for s, g in enumerate(GS):
        denom = B * A * g * g
        loss += (tot[2 * s] - tot[2 * s + 1]) / denom
    return np.float32(loss)


def _run_hw(in_maps, trace=False):
    from concourse.bass_utils import run_bass_kernel_spmd

    nc = _build()
    br = run_bass_kernel_spmd(nc, in_maps, list(range(NCORES)), trace=trace)
    return br


def kernel(out0, out1, out2, anchors0, anchors1, anchors2, targets):
    in_maps = _make_in_maps(
        out0, out1, out2, anchors0, anchors1, anchors2, targets
    )
    br = _run_hw(in_maps, trace=False)
    partials = [r["partial"] for r in br.results]
    return np.asarray(_reduce_partials(partials), dtype=np.float32)
